# revision 1
# baseline (speedup 1.0000x reference)
"""Trainium2 Bass kernel for nn_ComplexFFTRadNet (complex CNN, 4 cconv+BN+ReLU
layers, |.| magnitude, two 3x3 conv heads, sigmoid on cls channel).

Sharding: 8 NeuronCores = batch(4) x H-halves(2). Each core computes 64 output
rows of one image. Bottom halves are vertically FLIPPED on the host (input rows
and conv-kernel dy both reversed) so that every core runs the identical SPMD
program: global image edge at local top, 5 rows of neighbor halo at local
bottom. BatchNorm statistics (training-style, over N,H,W) are computed locally
per channel with bn_stats/bn_aggr over each core's owned 64 rows and combined
with a tiny AllReduce per layer.

Convolution = 9 shifted-window matmuls accumulated in PSUM; channels on the
partition (contraction) axis; activations stored row-major [C, H, W+2] with
zero pad columns so all 9 taps are contiguous-offset reads of one SBUF tile.
Real/imag are stacked along channels, so a complex conv is one real conv with
the block weight matrix [[wr, -wi], [wi, wr]].
"""
import os
import sys
import numpy as np
from contextlib import ExitStack

sys.path.insert(0, "/opt/trn_rl_repo")

from concourse import bass, bass_utils, tile, mybir, bacc  # noqa: E402

try:
    import ml_dtypes
    _BF16 = ml_dtypes.bfloat16
except Exception:  # pragma: no cover
    _BF16 = None

N_CORES = 8
H, W = 128, 224
WB = W + 2          # padded width
OWN = 64            # owned rows per core
R = 8               # output rows per chunk
CNT_LOCAL = float(OWN * W)          # elements per channel per core
CNT_TOTAL = float(4 * H * W)        # elements per channel globally
BN_EPS = 1e-5

# matmul/storage dtype: "f32" (exact) or "bf16" (4x faster PE, ~0.5% err)
DT_MODE = os.environ.get("KERNEL_DT", "bf16")

# layer table: (n_kg_load, kg_ch, K, n_kg_mm, n_mg, M_total, H_in_data, H_out)
#   L1: x[256] -> stacked 288 (yr144,yi144), kgroups 2x128, mgroups 3x96
#   L2: 288 -> 192, kgroups 3x96, mgroups 2x96
#   L3, L4: 192 -> 192, kgroups 2x96, mgroups 2x96
#   L5 head: mag[96] -> 3
LAYERS = [
    dict(K=128, n_kg=2, Cin=256, n_mg=3, Mg=96, Mtot=288, Hin=69, Hout=68),
    dict(K=96, n_kg=3, Cin=288, n_mg=2, Mg=96, Mtot=192, Hin=68, Hout=67),
    dict(K=96, n_kg=2, Cin=192, n_mg=2, Mg=96, Mtot=192, Hin=67, Hout=66),
    dict(K=96, n_kg=2, Cin=192, n_mg=2, Mg=96, Mtot=192, Hin=66, Hout=65),
    dict(K=96, n_kg=2, Cin=192, n_mg=1, Mg=3, Mtot=3, Hin=65, Hout=64),
]

_nc_cache = {}


def _dt(mode):
    return mybir.dt.bfloat16 if mode == "bf16" else mybir.dt.float32


def _npdt(mode):
    return _BF16 if mode == "bf16" else np.float32


def build_program(mode):
    if mode in _nc_cache:
        return _nc_cache[mode]
    DT = _dt(mode)
    F32 = mybir.dt.float32
    R = 16 if mode == "bf16" else 8  # chunk rows (SBUF-limited for f32)
    nc = bacc.Bacc("TRN2", target_bir_lowering=False, debug=False,
                   num_devices=N_CORES)

    # ---- external I/O ----
    x_ext = nc.dram_tensor("x", [256, 70, WB], DT, kind="ExternalInput").ap()
    w_ext = []
    for li, L in enumerate(LAYERS):
        w_ext.append(nc.dram_tensor(
            f"w{li + 1}", [L["K"], 9, L["n_kg"] if li != 4 else 1, L["Mtot"]],
            DT, kind="ExternalInput").ap())
    gb_ext = []
    for li in range(4):
        gb_ext.append(nc.dram_tensor(
            f"gb{li + 1}", [LAYERS[li]["Mtot"], 2], F32,
            kind="ExternalInput").ap())
    hb_ext = nc.dram_tensor("hb", [3, 1], F32, kind="ExternalInput").ap()
    out_ext = nc.dram_tensor("out", [3, OWN, W], F32,
                             kind="ExternalOutput").ap()

    FLAT = (R + 2) * WB + 2  # flat in-tile size (1 lead + rows+2 + 1 tail)

    with tile.TileContext(nc) as tc, ExitStack() as ctx:
        wpool = ctx.enter_context(tc.tile_pool(name="wts", bufs=2))
        inpool = ctx.enter_context(tc.tile_pool(name="inp", bufs=2))
        stpool = ctx.enter_context(tc.tile_pool(name="stage", bufs=3))
        pspool = ctx.enter_context(tc.tile_pool(name="ps", bufs=8, space="PSUM"))
        stats = ctx.enter_context(tc.tile_pool(name="stats", bufs=1))
        small = ctx.enter_context(tc.tile_pool(name="small", bufs=4))
        stp = ctx.enter_context(tc.tile_pool(name="stv", bufs=2))
        dram = ctx.enter_context(tc.tile_pool(name="dram", bufs=1, space="DRAM"))

        # constants
        eps_t = small.tile([128, 1], F32, tag="eps")
        nc.vector.memset(eps_t[:], BN_EPS)
        hb_t = small.tile([3, 1], F32, tag="hb")
        nc.sync.dma_start(out=hb_t[:], in_=hb_ext)

        # per-layer weights resident whole kernel
        w_t = []
        for li, L in enumerate(LAYERS):
            nkg = L["n_kg"] if li != 4 else 1
            t = wpool.tile([L["K"], 9, nkg, L["Mtot"]], DT, tag="w",
                           name=f"wt{li}")
            nc.sync.dma_start(out=t[:], in_=w_ext[li])
            w_t.append(t)

        # DRAM spill buffers for layer outputs (raw conv out, pre-BN)
        y_dram = []
        for li in range(4):
            L = LAYERS[li]
            y_dram.append(dram.tile([L["Mtot"], L["Hout"], WB], DT,
                                    tag=f"y{li}", name=f"y{li}"))
        cc_in = [dram.tile([LAYERS[li]["Mtot"], 2], F32, tag=f"cci{li}",
                           name=f"cci{li}")
                 for li in range(4)]
        cc_out = [dram.tile([LAYERS[li]["Mtot"], 2], F32, tag=f"cco{li}",
                            name=f"cco{li}")
                  for li in range(4)]

        st_cur = None  # list per kgroup of [96,2] tiles (s=col0, t=col1)

        for li, L in enumerate(LAYERS):
            K, Mg, Mtot, Hout, Hin = L["K"], L["Mg"], L["Mtot"], L["Hout"], L["Hin"]
            n_kg_load = L["n_kg"]
            is_head = li == 4
            n_mm_kg = 1 if is_head else n_kg_load
            n_chunks = (Hout + R - 1) // R

            # per-mg stats buffers [Mg, 64, 6]
            if not is_head:
                stat_t = [stats.tile([Mg, OWN, 6], F32, tag=f"sb{mg}",
                                     name=f"sb{li}_{mg}")
                          for mg in range(L["n_mg"])]

            for c in range(n_chunks):
                y0 = c * R
                rows = min(R, Hout - y0)
                used = (rows + 2) * WB  # data region size (from flat idx 1)
                tail = used + 1

                # ---- load input chunk per kgroup ----
                in_t = []
                for kg in range(n_kg_load):
                    it = inpool.tile([K if li == 0 else 96, FLAT], DT,
                                     tag=f"in{kg}")
                    nc.vector.memset(it[:, 0:1], 0.0)
                    nc.vector.memset(it[:, tail:tail + 1], 0.0)
                    if li == 0:
                        ch0 = kg * 128
                        nc.sync.dma_start(
                            out=it[:, 1:1 + used],
                            in_=x_ext[ch0:ch0 + 128, y0:y0 + rows + 2, :])
                    else:
                        ch0 = kg * 96
                        src = y_dram[li - 1]
                        if y0 == 0:
                            nc.vector.memset(it[:, 1:1 + WB], 0.0)
                            nc.sync.dma_start(
                                out=it[:, 1 + WB:1 + used],
                                in_=src[ch0:ch0 + 96, 0:rows + 1, :])
                            na, nb = 1 + WB, 1 + used
                        else:
                            nc.sync.dma_start(
                                out=it[:, 1:1 + used],
                                in_=src[ch0:ch0 + 96, y0 - 1:y0 + rows + 1, :])
                            na, nb = 1, 1 + used
                        # normalize + relu (BN of previous layer), in place
                        nc.scalar.activation(
                            out=it[:, na:nb], in_=it[:, na:nb],
                            func=mybir.ActivationFunctionType.Relu,
                            bias=st_cur[kg][:, 1:2], scale=st_cur[kg][:, 0:1])
                        # zero the W pad columns (post-normalize)
                        v3 = it[:, 1:1 + used].rearrange(
                            "p (r w) -> p r w", w=WB)
                        nc.vector.memset(v3[:, :, 0:1], 0.0)
                        nc.vector.memset(v3[:, :, WB - 1:WB], 0.0)
                    in_t.append(it)

                # ---- head: magnitude sqrt(re^2+im^2) ----
                if is_head:
                    mag = inpool.tile([96, FLAT], DT, tag="in2")
                    lim = tail + 1
                    nc.vector.tensor_mul(mag[:, 0:lim], in_t[0][:, 0:lim],
                                         in_t[0][:, 0:lim])
                    # square imag in place (it has no further readers)
                    nc.vector.tensor_mul(in_t[1][:, 0:lim], in_t[1][:, 0:lim],
                                         in_t[1][:, 0:lim])
                    nc.vector.tensor_add(mag[:, 0:lim], mag[:, 0:lim],
                                         in_t[1][:, 0:lim])
                    nc.scalar.activation(
                        out=mag[:, 0:lim], in_=mag[:, 0:lim],
                        func=mybir.ActivationFunctionType.Sqrt)
                    mm_in = [mag]
                else:
                    mm_in = in_t

                # ---- matmul tiles: 2 output rows per PSUM tile ----
                n_t = (rows + 1) // 2
                for mg in range(L["n_mg"]):
                    m0 = mg * Mg
                    stg = stpool.tile([Mg, R * WB], F32 if is_head else DT,
                                      tag="st")
                    for j in range(n_t):
                        r2 = min(2, rows - 2 * j)
                        N = r2 * WB
                        ps = pspool.tile([Mg, N], F32, tag="ps")
                        nmm = 9 * n_mm_kg
                        i_mm = 0
                        for kg in range(n_mm_kg):
                            for t in range(9):
                                dy, dx = t // 3 - 1, t % 3 - 1
                                off = 1 + (2 * j + 1 + dy) * WB + dx
                                nc.tensor.matmul(
                                    ps[:],
                                    w_t[li][:, t, kg, m0:m0 + Mg],
                                    mm_in[kg][:, off:off + N],
                                    start=(i_mm == 0), stop=(i_mm == nmm - 1))
                                i_mm += 1
                        if not is_head and y0 < OWN:
                            psv = ps[:].rearrange("p (r w) -> p r w", w=WB)
                            slot = y0 + 2 * j
                            for r in range(r2):
                                nc.vector.bn_stats(
                                    out=stat_t[mg][:, slot + r:slot + r + 1, :],
                                    in_=psv[:, r:r + 1, 1:1 + W])
                        dst = stg[:, 2 * j * WB:2 * j * WB + N]
                        if is_head:
                            nc.vector.tensor_scalar_add(
                                out=dst, in0=ps[:], scalar1=hb_t[:])
                        else:
                            nc.vector.tensor_copy(out=dst, in_=ps[:])
                    if is_head:
                        nc.scalar.activation(
                            out=stg[0:1, 0:rows * WB], in_=stg[0:1, 0:rows * WB],
                            func=mybir.ActivationFunctionType.Sigmoid)
                        sv = stg[:, 0:rows * WB].rearrange(
                            "p (r w) -> p r w", w=WB)
                        nc.sync.dma_start(
                            out=out_ext[:, y0:y0 + rows, :],
                            in_=sv[:, :, 1:1 + W])
                    else:
                        nc.sync.dma_start(
                            out=y_dram[li][m0:m0 + Mg, y0:y0 + rows, :],
                            in_=stg[:, 0:rows * WB])

            # ---- BN stats: aggregate, all-reduce, make scale/shift ----
            if not is_head:
                for mg in range(L["n_mg"]):
                    m0 = mg * Mg
                    mv = small.tile([Mg, 2], F32, tag="mv")
                    nc.vector.bn_aggr(out=mv[:], in_=stat_t[mg][:])
                    sums = small.tile([Mg, 2], F32, tag="sums")
                    nc.vector.tensor_scalar_mul(
                        out=sums[:, 0:1], in0=mv[:, 0:1], scalar1=CNT_LOCAL)
                    sq = small.tile([Mg, 1], F32, tag="sq")
                    nc.vector.tensor_mul(sq[:], mv[:, 0:1], mv[:, 0:1])
                    nc.vector.tensor_add(sq[:], sq[:], mv[:, 1:2])
                    nc.vector.tensor_scalar_mul(
                        out=sums[:, 1:2], in0=sq[:], scalar1=CNT_LOCAL)
                    nc.sync.dma_start(out=cc_in[li][m0:m0 + Mg, :],
                                      in_=sums[:])
                nc.gpsimd.collective_compute(
                    "AllReduce", mybir.AluOpType.add,
                    replica_groups=[list(range(N_CORES))],
                    ins=[cc_in[li][:].opt()], outs=[cc_out[li][:].opt()])
                # consumer kgroups of the next layer read 96-channel slices
                nL = LAYERS[li + 1]
                st_cur = []
                for kg in range(nL["n_kg"]):
                    k0 = kg * 96
                    sr = small.tile([96, 2], F32, tag="sr")
                    nc.sync.dma_start(out=sr[:], in_=cc_out[li][k0:k0 + 96, :])
                    gbt = small.tile([96, 2], F32, tag="gbt")
                    nc.sync.dma_start(out=gbt[:], in_=gb_ext[li][k0:k0 + 96, :])
                    mean = small.tile([96, 1], F32, tag="mean")
                    nc.vector.tensor_scalar_mul(
                        out=mean[:], in0=sr[:, 0:1], scalar1=1.0 / CNT_TOTAL)
                    var = small.tile([96, 1], F32, tag="var")
                    nc.vector.tensor_scalar_mul(
                        out=var[:], in0=sr[:, 1:2], scalar1=1.0 / CNT_TOTAL)
                    msq = small.tile([96, 1], F32, tag="msq")
                    nc.vector.tensor_mul(msq[:], mean[:], mean[:])
                    nc.vector.tensor_sub(var[:], var[:], msq[:])
                    std = small.tile([96, 1], F32, tag="std")
                    nc.scalar.activation(
                        out=std[:], in_=var[:],
                        func=mybir.ActivationFunctionType.Sqrt,
                        bias=eps_t[0:96, :])
                    rstd = small.tile([96, 1], F32, tag="rstd")
                    nc.vector.reciprocal(out=rstd[:], in_=std[:])
                    st = stp.tile([96, 2], F32, tag=f"stv{kg}")
                    nc.vector.tensor_mul(st[:, 0:1], rstd[:], gbt[:, 0:1])
                    tmp2 = small.tile([96, 1], F32, tag="tmp2")
                    nc.vector.tensor_mul(tmp2[:], mean[:], st[:, 0:1])
                    nc.vector.tensor_sub(st[:, 1:2], gbt[:, 1:2], tmp2[:])
                    st_cur.append(st)

    nc.compile()
    _nc_cache[mode] = nc
    return nc


def _prep_inputs(x, w1r, w1i, g1, b1, w2r, w2i, g2, b2,
                 w3r, w3i, g3, b3, w4r, w4i, g4, b4, wc, bc, wg, bg,
                 mode):
    """Host-side shard + pack. Returns in_maps list of 8 dicts."""
    npdt = _npdt(mode)
    x = np.asarray(x, np.float32)

    # stacked block weights [Mtot, Cin, 3, 3]
    W1 = np.concatenate([w1r, w1i], axis=0)
    def blk(wr, wi):
        top = np.concatenate([wr, -wi], axis=1)
        bot = np.concatenate([wi, wr], axis=1)
        return np.concatenate([top, bot], axis=0)
    W2, W3, W4 = blk(w2r, w2i), blk(w3r, w3i), blk(w4r, w4i)
    W5 = np.concatenate([wc, wg], axis=0)
    Ws = [W1, W2, W3, W4, W5]

    def pack_w(Wf, K, nkg, flip):
        # -> [K, 9, nkg, Mtot] with t = ky*3+kx, k-groups along Cin
        if flip:
            Wf = Wf[:, :, ::-1, :]
        Mtot, Cin = Wf.shape[0], Wf.shape[1]
        a = Wf.transpose(2, 3, 1, 0).reshape(9, Cin, Mtot)  # [t, cin, m]
        a = a.reshape(9, nkg, K, Mtot).transpose(2, 0, 1, 3)  # [K,9,g,M]
        return np.ascontiguousarray(a, dtype=npdt)

    gbs = []
    for g, b in ((g1, b1), (g2, b2), (g3, b3), (g4, b4)):
        gs = np.concatenate([g, g]).astype(np.float32)
        bs = np.concatenate([b, b]).astype(np.float32)
        gbs.append(np.ascontiguousarray(np.stack([gs, bs], axis=1)))
    hb = np.concatenate([bc, bg]).astype(np.float32).reshape(3, 1)

    in_maps = []
    for core in range(N_CORES):
        b_idx, h = core // 2, core % 2
        xi = x[b_idx]
        if h == 1:
            xi = xi[:, ::-1, :]
        # x_shard [256, 70, WB]: row 0 zero (local -1), rows 1..69 = local 0..68
        xs = np.zeros((256, 70, WB), np.float32)
        xs[:, 1:70, 1:1 + W] = xi[:, 0:69, :]
        m = {"x": xs.astype(npdt), "hb": hb}
        for li, L in enumerate(LAYERS):
            nkg = L["n_kg"] if li != 4 else 1
            m[f"w{li + 1}"] = pack_w(Ws[li], L["K"], nkg, flip=(h == 1))
        for li in range(4):
            m[f"gb{li + 1}"] = gbs[li]
        in_maps.append(m)
    return in_maps


_runner_cache = {}


def _get_runner(mode):
    """Build the SPMD jit executable once; returns run(in_maps) -> list of
    per-core output dicts. Mirrors bass2jax.run_bass_via_pjrt but caches the
    jitted callable so repeated kernel() calls don't re-trace/re-compile."""
    if mode in _runner_cache:
        return _runner_cache[mode]
    import jax
    from concourse import bass2jax
    from jax.experimental.shard_map import shard_map
    from jax.sharding import Mesh, PartitionSpec

    nc = build_program(mode)
    bass2jax.install_neuronx_cc_hook()

    partition_name = (nc.partition_id_tensor.name
                      if nc.partition_id_tensor else None)
    in_names, out_names, out_avals, zero_outs = [], [], [], []
    for alloc in nc.m.functions[0].allocations:
        if not isinstance(alloc, mybir.MemoryLocationSet):
            continue
        name = alloc.memorylocations[0].name
        if alloc.kind == "ExternalInput":
            if name != partition_name:
                in_names.append(name)
        elif alloc.kind == "ExternalOutput":
            shape = tuple(alloc.tensor_shape)
            dtype = mybir.dt.np(alloc.dtype)
            out_names.append(name)
            out_avals.append(jax.core.ShapedArray(shape, dtype))
            zero_outs.append(np.zeros(shape, dtype))
    n_params, n_outs = len(in_names), len(out_avals)
    all_names = list(in_names + out_names)
    if partition_name is not None:
        all_names.append(partition_name)
    all_names = tuple(all_names)
    donate = tuple(range(n_params, n_params + n_outs))

    def _body(*args):
        operands = list(args)
        if partition_name is not None:
            operands.append(bass2jax.partition_id_tensor())
        outs = bass2jax._bass_exec_p.bind(
            *operands,
            out_avals=tuple(out_avals),
            in_names=all_names,
            out_names=tuple(out_names),
            lowering_input_output_aliases=(),
            sim_require_finite=True,
            sim_require_nnan=True,
            nc=nc,
        )
        return tuple(outs)

    devices = jax.devices()[:N_CORES]
    mesh = Mesh(np.asarray(devices), ("core",))
    in_specs = (PartitionSpec("core"),) * (n_params + n_outs)
    out_specs = (PartitionSpec("core"),) * n_outs
    sharded = jax.jit(
        shard_map(_body, mesh=mesh, in_specs=in_specs, out_specs=out_specs,
                  check_rep=False),
        donate_argnums=donate, keep_unused=True)

    def run(in_maps):
        concat_in = [
            np.concatenate([np.asarray(in_maps[c][nm]) for c in
                            range(N_CORES)], axis=0)
            for nm in in_names
        ]
        concat_zeros = [
            np.zeros((N_CORES * z.shape[0], *z.shape[1:]), z.dtype)
            for z in zero_outs
        ]
        out_arrs = sharded(*concat_in, *concat_zeros)
        return [
            {nm: np.asarray(out_arrs[i]).reshape(N_CORES, *out_avals[i].shape)[c]
             for i, nm in enumerate(out_names)}
            for c in range(N_CORES)
        ]

    def time_device(in_maps, reps=5):
        """Time executions with inputs pre-staged on device (excludes host
        prep and host->device transfer). Returns list of seconds."""
        import time as _time
        from jax.sharding import NamedSharding
        concat_in = [
            np.concatenate([np.asarray(in_maps[c][nm]) for c in
                            range(N_CORES)], axis=0)
            for nm in in_names
        ]
        sh = NamedSharding(mesh, PartitionSpec("core"))
        dev_in = [jax.device_put(a, sh) for a in concat_in]
        for a in dev_in:
            a.block_until_ready()
        times = []
        for _ in range(reps):
            concat_zeros = [
                jax.device_put(
                    np.zeros((N_CORES * z.shape[0], *z.shape[1:]), z.dtype),
                    sh)
                for z in zero_outs
            ]
            for a in concat_zeros:
                a.block_until_ready()
            t0 = _time.time()
            out_arrs = sharded(*dev_in, *concat_zeros)
            for o in out_arrs:
                o.block_until_ready()
            times.append(_time.time() - t0)
        return times

    run.time_device = time_device
    _runner_cache[mode] = run
    return run


def kernel(**inputs):
    mode = DT_MODE
    run = _get_runner(mode)
    in_maps = _prep_inputs(mode=mode, **inputs)
    results = run(in_maps)
    out = np.zeros((4, 3, H, W), np.float32)
    for core in range(N_CORES):
        b_idx, h = core // 2, core % 2
        oc = results[core]["out"]  # [3, 64, W]
        if h == 0:
            out[b_idx, :, 0:OWN, :] = oc
        else:
            out[b_idx, :, OWN:H, :] = oc[:, ::-1, :]
    return out



# revision 4
# speedup vs baseline: 49.8249x; 49.8249x over previous
"""Trainium2 Bass kernel for nn_ComplexFFTRadNet (complex CNN, 4 cconv+BN+ReLU
layers, |.| magnitude, two 3x3 conv heads, sigmoid on cls channel).

Sharding: 8 NeuronCores = batch(4) x H-halves(2). Each core computes 64 output
rows of one image. Bottom halves are vertically FLIPPED on the host (input rows
and conv-kernel dy both reversed) so that every core runs the identical SPMD
program: global image edge at local top, 5 rows of neighbor halo at local
bottom. BatchNorm statistics (training-style, over N,H,W) are computed locally
per channel with bn_stats/bn_aggr over each core's owned 64 rows and combined
with a tiny AllReduce per layer.

Convolution = 9 shifted-window matmuls accumulated in PSUM; channels on the
partition (contraction) axis; activations stored row-major [C, H, W+2] with
zero pad columns so all 9 taps are contiguous-offset reads of one SBUF tile.
Real/imag are stacked along channels, so a complex conv is one real conv with
the block weight matrix [[wr, -wi], [wi, wr]].
"""
import os
import sys
import numpy as np
from contextlib import ExitStack

sys.path.insert(0, "/opt/trn_rl_repo")

from concourse import bass, bass_utils, tile, mybir, bacc  # noqa: E402

try:
    import ml_dtypes
    _BF16 = ml_dtypes.bfloat16
except Exception:  # pragma: no cover
    _BF16 = None

N_CORES = 8
H, W = 128, 224
WB = W + 2          # padded width
OWN = 64            # owned rows per core
R = 8               # output rows per chunk
CNT_LOCAL = float(OWN * W)          # elements per channel per core
CNT_TOTAL = float(4 * H * W)        # elements per channel globally
BN_EPS = 1e-5

# matmul/storage dtype: "f32" (exact) or "bf16" (4x faster PE, ~0.5% err)
DT_MODE = os.environ.get("KERNEL_DT", "bf16")

# layer table: (n_kg_load, kg_ch, K, n_kg_mm, n_mg, M_total, H_in_data, H_out)
#   L1: x[256] -> stacked 288 (yr144,yi144), kgroups 2x128, mgroups 3x96
#   L2: 288 -> 192, kgroups 3x96, mgroups 2x96
#   L3, L4: 192 -> 192, kgroups 2x96, mgroups 2x96
#   L5 head: mag[96] -> 3
LAYERS = [
    dict(K=128, n_kg=2, Cin=256, n_mg=3, Mg=96, Mtot=288, Hin=69, Hout=68),
    dict(K=96, n_kg=3, Cin=288, n_mg=2, Mg=96, Mtot=192, Hin=68, Hout=67),
    dict(K=96, n_kg=2, Cin=192, n_mg=2, Mg=96, Mtot=192, Hin=67, Hout=66),
    dict(K=96, n_kg=2, Cin=192, n_mg=2, Mg=96, Mtot=192, Hin=66, Hout=65),
    dict(K=96, n_kg=2, Cin=192, n_mg=1, Mg=3, Mtot=3, Hin=65, Hout=64),
]

_nc_cache = {}


def _dt(mode):
    return mybir.dt.bfloat16 if mode == "bf16" else mybir.dt.float32


def _npdt(mode):
    return _BF16 if mode == "bf16" else np.float32


def build_program(mode):
    if mode in _nc_cache:
        return _nc_cache[mode]
    DT = _dt(mode)
    F32 = mybir.dt.float32
    R = 16 if mode == "bf16" else 8  # chunk rows (SBUF-limited for f32)
    nc = bacc.Bacc("TRN2", target_bir_lowering=False, debug=False,
                   num_devices=N_CORES)

    # ---- external I/O ----
    x_ext = nc.dram_tensor("x", [256, 70, WB], DT, kind="ExternalInput").ap()
    w_ext = []
    for li, L in enumerate(LAYERS):
        w_ext.append(nc.dram_tensor(
            f"w{li + 1}", [L["K"], 9, L["n_kg"] if li != 4 else 1, L["Mtot"]],
            DT, kind="ExternalInput").ap())
    gb_ext = []
    for li in range(4):
        gb_ext.append(nc.dram_tensor(
            f"gb{li + 1}", [LAYERS[li]["Mtot"], 2], F32,
            kind="ExternalInput").ap())
    hb_ext = nc.dram_tensor("hb", [3, 1], F32, kind="ExternalInput").ap()
    out_ext = nc.dram_tensor("out", [3, OWN, W], F32,
                             kind="ExternalOutput").ap()

    FLAT = (R + 2) * WB + 2  # flat in-tile size (1 lead + rows+2 + 1 tail)

    with tile.TileContext(nc) as tc, ExitStack() as ctx:
        wpool = ctx.enter_context(tc.tile_pool(name="wts", bufs=2))
        inpool = ctx.enter_context(tc.tile_pool(name="inp", bufs=2))
        stpool = ctx.enter_context(tc.tile_pool(name="stage", bufs=3))
        pspool = ctx.enter_context(tc.tile_pool(name="ps", bufs=8, space="PSUM"))
        stats = ctx.enter_context(tc.tile_pool(name="stats", bufs=1))
        small = ctx.enter_context(tc.tile_pool(name="small", bufs=4))
        stp = ctx.enter_context(tc.tile_pool(name="stv", bufs=2))
        dram = ctx.enter_context(tc.tile_pool(name="dram", bufs=1, space="DRAM"))

        # constants
        eps_t = small.tile([128, 1], F32, tag="eps")
        nc.vector.memset(eps_t[:], BN_EPS)
        hb_t = small.tile([3, 1], F32, tag="hb")
        nc.sync.dma_start(out=hb_t[:], in_=hb_ext)

        # per-layer weights resident whole kernel
        w_t = []
        for li, L in enumerate(LAYERS):
            nkg = L["n_kg"] if li != 4 else 1
            t = wpool.tile([L["K"], 9, nkg, L["Mtot"]], DT, tag="w",
                           name=f"wt{li}")
            nc.sync.dma_start(out=t[:], in_=w_ext[li])
            w_t.append(t)

        # DRAM spill buffers for layer outputs (raw conv out, pre-BN)
        y_dram = []
        for li in range(4):
            L = LAYERS[li]
            y_dram.append(dram.tile([L["Mtot"], L["Hout"], WB], DT,
                                    tag=f"y{li}", name=f"y{li}"))
        cc_in = [dram.tile([LAYERS[li]["Mtot"], 2], F32, tag=f"cci{li}",
                           name=f"cci{li}")
                 for li in range(4)]
        cc_out = [dram.tile([LAYERS[li]["Mtot"], 2], F32, tag=f"cco{li}",
                            name=f"cco{li}")
                  for li in range(4)]

        st_cur = None  # list per kgroup of [96,2] tiles (s=col0, t=col1)

        def emit_stats(li, L, stat_t):
            """Aggregate BN stats, AllReduce, and build scale/shift for the
            next layer's kgroups. Issued right after the last owned-row chunk
            so the collective overlaps the halo-row matmuls."""
            Mg = L["Mg"]
            for mg in range(L["n_mg"]):
                m0 = mg * Mg
                mv = small.tile([Mg, 2], F32, tag="mv")
                nc.vector.bn_aggr(out=mv[:], in_=stat_t[mg][:])
                sums = small.tile([Mg, 2], F32, tag="sums")
                nc.vector.tensor_scalar_mul(
                    out=sums[:, 0:1], in0=mv[:, 0:1], scalar1=CNT_LOCAL)
                sq = small.tile([Mg, 1], F32, tag="sq")
                nc.vector.tensor_mul(sq[:], mv[:, 0:1], mv[:, 0:1])
                nc.vector.tensor_add(sq[:], sq[:], mv[:, 1:2])
                nc.vector.tensor_scalar_mul(
                    out=sums[:, 1:2], in0=sq[:], scalar1=CNT_LOCAL)
                nc.sync.dma_start(out=cc_in[li][m0:m0 + Mg, :], in_=sums[:])
            nc.gpsimd.collective_compute(
                "AllReduce", mybir.AluOpType.add,
                replica_groups=[list(range(N_CORES))],
                ins=[cc_in[li][:].opt()], outs=[cc_out[li][:].opt()])
            # consumer kgroups of the next layer read 96-channel slices
            nL = LAYERS[li + 1]
            st_next = []
            for kg in range(nL["n_kg"]):
                k0 = kg * 96
                sr = small.tile([96, 2], F32, tag="sr")
                nc.sync.dma_start(out=sr[:], in_=cc_out[li][k0:k0 + 96, :])
                gbt = small.tile([96, 2], F32, tag="gbt")
                nc.sync.dma_start(out=gbt[:], in_=gb_ext[li][k0:k0 + 96, :])
                mean = small.tile([96, 1], F32, tag="mean")
                nc.vector.tensor_scalar_mul(
                    out=mean[:], in0=sr[:, 0:1], scalar1=1.0 / CNT_TOTAL)
                var = small.tile([96, 1], F32, tag="var")
                nc.vector.tensor_scalar_mul(
                    out=var[:], in0=sr[:, 1:2], scalar1=1.0 / CNT_TOTAL)
                msq = small.tile([96, 1], F32, tag="msq")
                nc.vector.tensor_mul(msq[:], mean[:], mean[:])
                nc.vector.tensor_sub(var[:], var[:], msq[:])
                std = small.tile([96, 1], F32, tag="std")
                nc.scalar.activation(
                    out=std[:], in_=var[:],
                    func=mybir.ActivationFunctionType.Sqrt,
                    bias=eps_t[0:96, :])
                rstd = small.tile([96, 1], F32, tag="rstd")
                nc.vector.reciprocal(out=rstd[:], in_=std[:])
                st = stp.tile([96, 2], F32, tag=f"stv{kg}")
                nc.vector.tensor_mul(st[:, 0:1], rstd[:], gbt[:, 0:1])
                tmp2 = small.tile([96, 1], F32, tag="tmp2")
                nc.vector.tensor_mul(tmp2[:], mean[:], st[:, 0:1])
                nc.vector.tensor_sub(st[:, 1:2], gbt[:, 1:2], tmp2[:])
                st_next.append(st)
            return st_next

        for li, L in enumerate(LAYERS):
            K, Mg, Mtot, Hout, Hin = L["K"], L["Mg"], L["Mtot"], L["Hout"], L["Hin"]
            n_kg_load = L["n_kg"]
            is_head = li == 4
            n_mm_kg = 1 if is_head else n_kg_load
            n_chunks = (Hout + R - 1) // R
            # last chunk whose rows contribute BN stats (owned rows 0..63);
            # the stats chain + AllReduce issue right after it so the
            # collective overlaps the remaining halo-row chunks.
            own_last = (OWN + R - 1) // R - 1
            st_next = None

            # per-mg stats buffers [Mg, 64, 6]
            if not is_head:
                stat_t = [stats.tile([Mg, OWN, 6], F32, tag=f"sb{mg}",
                                     name=f"sb{li}_{mg}")
                          for mg in range(L["n_mg"])]

            for c in range(n_chunks):
                y0 = c * R
                rows = min(R, Hout - y0)
                used = (rows + 2) * WB  # data region size (from flat idx 1)
                tail = used + 1

                # ---- load input chunk per kgroup ----
                in_t = []
                for kg in range(n_kg_load):
                    it = inpool.tile([K if li == 0 else 96, FLAT], DT,
                                     tag=f"in{kg}")
                    nc.vector.memset(it[:, 0:1], 0.0)
                    nc.vector.memset(it[:, tail:tail + 1], 0.0)
                    if li == 0:
                        ch0 = kg * 128
                        nc.sync.dma_start(
                            out=it[:, 1:1 + used],
                            in_=x_ext[ch0:ch0 + 128, y0:y0 + rows + 2, :])
                    else:
                        ch0 = kg * 96
                        src = y_dram[li - 1]
                        if y0 == 0:
                            nc.vector.memset(it[:, 1:1 + WB], 0.0)
                            nc.sync.dma_start(
                                out=it[:, 1 + WB:1 + used],
                                in_=src[ch0:ch0 + 96, 0:rows + 1, :])
                            na, nb = 1 + WB, 1 + used
                        else:
                            nc.sync.dma_start(
                                out=it[:, 1:1 + used],
                                in_=src[ch0:ch0 + 96, y0 - 1:y0 + rows + 1, :])
                            na, nb = 1, 1 + used
                        # normalize + relu (BN of previous layer), in place
                        nc.scalar.activation(
                            out=it[:, na:nb], in_=it[:, na:nb],
                            func=mybir.ActivationFunctionType.Relu,
                            bias=st_cur[kg][:, 1:2], scale=st_cur[kg][:, 0:1])
                        # zero the W pad columns (post-normalize)
                        v3 = it[:, 1:1 + used].rearrange(
                            "p (r w) -> p r w", w=WB)
                        nc.vector.memset(v3[:, :, 0:1], 0.0)
                        nc.vector.memset(v3[:, :, WB - 1:WB], 0.0)
                    in_t.append(it)

                # ---- head: magnitude sqrt(re^2+im^2) ----
                if is_head:
                    mag = inpool.tile([96, FLAT], DT, tag="in2")
                    lim = tail + 1
                    nc.vector.tensor_mul(mag[:, 0:lim], in_t[0][:, 0:lim],
                                         in_t[0][:, 0:lim])
                    # square imag in place (it has no further readers)
                    nc.vector.tensor_mul(in_t[1][:, 0:lim], in_t[1][:, 0:lim],
                                         in_t[1][:, 0:lim])
                    nc.vector.tensor_add(mag[:, 0:lim], mag[:, 0:lim],
                                         in_t[1][:, 0:lim])
                    nc.scalar.activation(
                        out=mag[:, 0:lim], in_=mag[:, 0:lim],
                        func=mybir.ActivationFunctionType.Sqrt)
                    mm_in = [mag]
                else:
                    mm_in = in_t

                # ---- matmul tiles: 2 output rows per PSUM tile ----
                n_t = (rows + 1) // 2
                for mg in range(L["n_mg"]):
                    m0 = mg * Mg
                    stg = stpool.tile([Mg, R * WB], F32 if is_head else DT,
                                      tag="st")
                    for j in range(n_t):
                        r2 = min(2, rows - 2 * j)
                        N = r2 * WB
                        ps = pspool.tile([Mg, N], F32, tag="ps")
                        nmm = 9 * n_mm_kg
                        i_mm = 0
                        for kg in range(n_mm_kg):
                            for t in range(9):
                                dy, dx = t // 3 - 1, t % 3 - 1
                                off = 1 + (2 * j + 1 + dy) * WB + dx
                                nc.tensor.matmul(
                                    ps[:],
                                    w_t[li][:, t, kg, m0:m0 + Mg],
                                    mm_in[kg][:, off:off + N],
                                    start=(i_mm == 0), stop=(i_mm == nmm - 1))
                                i_mm += 1
                        if not is_head and y0 < OWN:
                            psv = ps[:].rearrange("p (r w) -> p r w", w=WB)
                            slot = y0 + 2 * j
                            for r in range(r2):
                                nc.vector.bn_stats(
                                    out=stat_t[mg][:, slot + r:slot + r + 1, :],
                                    in_=psv[:, r:r + 1, 1:1 + W])
                        dst = stg[:, 2 * j * WB:2 * j * WB + N]
                        if is_head:
                            nc.vector.tensor_scalar_add(
                                out=dst, in0=ps[:], scalar1=hb_t[:])
                        else:
                            nc.vector.tensor_copy(out=dst, in_=ps[:])
                    if is_head:
                        nc.scalar.activation(
                            out=stg[0:1, 0:rows * WB], in_=stg[0:1, 0:rows * WB],
                            func=mybir.ActivationFunctionType.Sigmoid)
                        sv = stg[:, 0:rows * WB].rearrange(
                            "p (r w) -> p r w", w=WB)
                        nc.sync.dma_start(
                            out=out_ext[:, y0:y0 + rows, :],
                            in_=sv[:, :, 1:1 + W])
                    else:
                        nc.sync.dma_start(
                            out=y_dram[li][m0:m0 + Mg, y0:y0 + rows, :],
                            in_=stg[:, 0:rows * WB])

                if not is_head and c == own_last:
                    st_next = emit_stats(li, L, stat_t)

            if not is_head:
                st_cur = st_next

    nc.compile()
    _nc_cache[mode] = nc
    return nc


def _prep_inputs(x, w1r, w1i, g1, b1, w2r, w2i, g2, b2,
                 w3r, w3i, g3, b3, w4r, w4i, g4, b4, wc, bc, wg, bg,
                 mode):
    """Host-side shard + pack. Returns in_maps list of 8 dicts."""
    npdt = _npdt(mode)
    x = np.asarray(x, np.float32)

    # stacked block weights [Mtot, Cin, 3, 3]
    W1 = np.concatenate([w1r, w1i], axis=0)
    def blk(wr, wi):
        top = np.concatenate([wr, -wi], axis=1)
        bot = np.concatenate([wi, wr], axis=1)
        return np.concatenate([top, bot], axis=0)
    W2, W3, W4 = blk(w2r, w2i), blk(w3r, w3i), blk(w4r, w4i)
    W5 = np.concatenate([wc, wg], axis=0)
    Ws = [W1, W2, W3, W4, W5]

    def pack_w(Wf, K, nkg, flip):
        # -> [K, 9, nkg, Mtot] with t = ky*3+kx, k-groups along Cin
        if flip:
            Wf = Wf[:, :, ::-1, :]
        Mtot, Cin = Wf.shape[0], Wf.shape[1]
        a = Wf.transpose(2, 3, 1, 0).reshape(9, Cin, Mtot)  # [t, cin, m]
        a = a.reshape(9, nkg, K, Mtot).transpose(2, 0, 1, 3)  # [K,9,g,M]
        return np.ascontiguousarray(a, dtype=npdt)

    gbs = []
    for g, b in ((g1, b1), (g2, b2), (g3, b3), (g4, b4)):
        gs = np.concatenate([g, g]).astype(np.float32)
        bs = np.concatenate([b, b]).astype(np.float32)
        gbs.append(np.ascontiguousarray(np.stack([gs, bs], axis=1)))
    hb = np.concatenate([bc, bg]).astype(np.float32).reshape(3, 1)

    in_maps = []
    for core in range(N_CORES):
        b_idx, h = core // 2, core % 2
        xi = x[b_idx]
        if h == 1:
            xi = xi[:, ::-1, :]
        # x_shard [256, 70, WB]: row 0 zero (local -1), rows 1..69 = local 0..68
        xs = np.zeros((256, 70, WB), np.float32)
        xs[:, 1:70, 1:1 + W] = xi[:, 0:69, :]
        m = {"x": xs.astype(npdt), "hb": hb}
        for li, L in enumerate(LAYERS):
            nkg = L["n_kg"] if li != 4 else 1
            m[f"w{li + 1}"] = pack_w(Ws[li], L["K"], nkg, flip=(h == 1))
        for li in range(4):
            m[f"gb{li + 1}"] = gbs[li]
        in_maps.append(m)
    return in_maps


_runner_cache = {}


def _get_runner(mode):
    """Build the SPMD jit executable once; returns run(in_maps) -> list of
    per-core output dicts. Mirrors bass2jax.run_bass_via_pjrt but caches the
    jitted callable so repeated kernel() calls don't re-trace/re-compile."""
    if mode in _runner_cache:
        return _runner_cache[mode]
    import jax
    from concourse import bass2jax
    from jax.experimental.shard_map import shard_map
    from jax.sharding import Mesh, PartitionSpec

    nc = build_program(mode)
    bass2jax.install_neuronx_cc_hook()

    partition_name = (nc.partition_id_tensor.name
                      if nc.partition_id_tensor else None)
    in_names, out_names, out_avals, zero_outs = [], [], [], []
    for alloc in nc.m.functions[0].allocations:
        if not isinstance(alloc, mybir.MemoryLocationSet):
            continue
        name = alloc.memorylocations[0].name
        if alloc.kind == "ExternalInput":
            if name != partition_name:
                in_names.append(name)
        elif alloc.kind == "ExternalOutput":
            shape = tuple(alloc.tensor_shape)
            dtype = mybir.dt.np(alloc.dtype)
            out_names.append(name)
            out_avals.append(jax.core.ShapedArray(shape, dtype))
            zero_outs.append(np.zeros(shape, dtype))
    n_params, n_outs = len(in_names), len(out_avals)
    all_names = list(in_names + out_names)
    if partition_name is not None:
        all_names.append(partition_name)
    all_names = tuple(all_names)
    donate = tuple(range(n_params, n_params + n_outs))

    def _body(*args):
        operands = list(args)
        if partition_name is not None:
            operands.append(bass2jax.partition_id_tensor())
        outs = bass2jax._bass_exec_p.bind(
            *operands,
            out_avals=tuple(out_avals),
            in_names=all_names,
            out_names=tuple(out_names),
            lowering_input_output_aliases=(),
            sim_require_finite=True,
            sim_require_nnan=True,
            nc=nc,
        )
        return tuple(outs)

    devices = jax.devices()[:N_CORES]
    mesh = Mesh(np.asarray(devices), ("core",))
    in_specs = (PartitionSpec("core"),) * (n_params + n_outs)
    out_specs = (PartitionSpec("core"),) * n_outs
    sharded = jax.jit(
        shard_map(_body, mesh=mesh, in_specs=in_specs, out_specs=out_specs,
                  check_rep=False),
        donate_argnums=donate, keep_unused=True)

    def run(in_maps):
        concat_in = [
            np.concatenate([np.asarray(in_maps[c][nm]) for c in
                            range(N_CORES)], axis=0)
            for nm in in_names
        ]
        concat_zeros = [
            np.zeros((N_CORES * z.shape[0], *z.shape[1:]), z.dtype)
            for z in zero_outs
        ]
        out_arrs = sharded(*concat_in, *concat_zeros)
        return [
            {nm: np.asarray(out_arrs[i]).reshape(N_CORES, *out_avals[i].shape)[c]
             for i, nm in enumerate(out_names)}
            for c in range(N_CORES)
        ]

    def time_device(in_maps, reps=5):
        """Time executions with inputs pre-staged on device (excludes host
        prep and host->device transfer). Returns list of seconds. NOTE: a
        single blocking dispatch through the axon tunnel carries ~80-90 ms
        of fixed round-trip latency that is NOT device execution time."""
        import time as _time
        from jax.sharding import NamedSharding
        concat_in = [
            np.concatenate([np.asarray(in_maps[c][nm]) for c in
                            range(N_CORES)], axis=0)
            for nm in in_names
        ]
        sh = NamedSharding(mesh, PartitionSpec("core"))
        dev_in = [jax.device_put(a, sh) for a in concat_in]
        for a in dev_in:
            a.block_until_ready()
        times = []
        for _ in range(reps):
            concat_zeros = [
                jax.device_put(
                    np.zeros((N_CORES * z.shape[0], *z.shape[1:]), z.dtype),
                    sh)
                for z in zero_outs
            ]
            for a in concat_zeros:
                a.block_until_ready()
            t0 = _time.time()
            out_arrs = sharded(*dev_in, *concat_zeros)
            for o in out_arrs:
                o.block_until_ready()
            times.append(_time.time() - t0)
        return times

    def time_device_pipelined(in_maps, Ks=(2, 34), reps=4):
        """Measure the true per-execution device time by chaining K identical
        executions (call i's outputs seed call i+1's donated output operands,
        creating a data dependency so executions run back-to-back on the
        device) and taking the wall-clock slope between two chain lengths.
        This amortizes the axon-tunnel round-trip latency (~85 ms, paid once
        per chain) to zero; the slope is the steady-state per-execution
        hardware time. Returns (t_exec_seconds, {K: best_wall_seconds})."""
        import time as _time
        from jax.sharding import NamedSharding
        concat_in = [
            np.concatenate([np.asarray(in_maps[c][nm]) for c in
                            range(N_CORES)], axis=0)
            for nm in in_names
        ]
        sh = NamedSharding(mesh, PartitionSpec("core"))
        dev_in = [jax.device_put(a, sh) for a in concat_in]
        for a in dev_in:
            a.block_until_ready()

        def make_zeros():
            return [
                jax.device_put(
                    np.zeros((N_CORES * z.shape[0], *z.shape[1:]), z.dtype),
                    sh)
                for z in zero_outs
            ]

        # warm-up execution
        outs = tuple(sharded(*dev_in, *make_zeros()))
        for o in outs:
            o.block_until_ready()
        walls = {}
        for K in sorted(Ks):
            ts = []
            for _ in range(reps):
                outs = tuple(make_zeros())
                for o in outs:
                    o.block_until_ready()
                t0 = _time.time()
                for _ in range(K):
                    outs = sharded(*dev_in, *outs)
                for o in outs:
                    o.block_until_ready()
                ts.append(_time.time() - t0)
            walls[K] = min(ts)
        K1, K2 = min(Ks), max(Ks)
        t_exec = (walls[K2] - walls[K1]) / (K2 - K1)
        return t_exec, walls

    run.time_device = time_device
    run.time_device_pipelined = time_device_pipelined
    _runner_cache[mode] = run
    return run


def kernel(**inputs):
    mode = DT_MODE
    run = _get_runner(mode)
    in_maps = _prep_inputs(mode=mode, **inputs)
    results = run(in_maps)
    out = np.zeros((4, 3, H, W), np.float32)
    for core in range(N_CORES):
        b_idx, h = core // 2, core % 2
        oc = results[core]["out"]  # [3, 64, W]
        if h == 0:
            out[b_idx, :, 0:OWN, :] = oc
        else:
            out[b_idx, :, OWN:H, :] = oc[:, ::-1, :]
    return out



# revision 11
# speedup vs baseline: 159.3320x; 3.1978x over previous
"""Trainium2 Bass kernel for nn_ComplexFFTRadNet (complex CNN, 4 cconv+BN+ReLU
layers, |.| magnitude, two 3x3 conv heads, sigmoid on cls channel).

Sharding: 8 NeuronCores = batch(4) x H-halves(2). Each core computes 64 output
rows of one image. Bottom halves are vertically FLIPPED on the host (input rows
and conv-kernel dy both reversed) so that every core runs the identical SPMD
program: global image edge at local top, 5 rows of neighbor halo at local
bottom. BatchNorm statistics (training-style, over N,H,W) are computed locally
per channel with bn_stats/bn_aggr over each core's owned 64 rows and combined
with a tiny AllReduce per layer.

Convolution = 9 shifted-window matmuls accumulated in PSUM; channels on the
partition (contraction) axis; activations stored row-major [C, H, W+2] with
zero pad columns so all 9 taps are contiguous-offset reads of one SBUF tile.
Real/imag are stacked along channels, so a complex conv is one real conv with
the block weight matrix [[wr, -wi], [wi, wr]].
"""
import os
import sys
import numpy as np
from contextlib import ExitStack

sys.path.insert(0, "/opt/trn_rl_repo")

from concourse import bass, bass_utils, tile, mybir, bacc  # noqa: E402

try:
    import ml_dtypes
    _BF16 = ml_dtypes.bfloat16
except Exception:  # pragma: no cover
    _BF16 = None

N_CORES = 8
H, W = 128, 224
WB = W + 2          # padded width
OWN = 64            # owned rows per core
R = 8               # output rows per chunk
CNT_LOCAL = float(OWN * W)          # elements per channel per core
CNT_TOTAL = float(4 * H * W)        # elements per channel globally
BN_EPS = 1e-5

# matmul/storage dtype: "f32" (exact) or "bf16" (4x faster PE, ~0.5% err)
DT_MODE = os.environ.get("KERNEL_DT", "bf16")
# emit BN stats + AllReduce right after the last owned-row chunk (overlaps
# the collective with halo-row matmuls) vs after the whole layer
OVERLAP_STATS = os.environ.get("KERNEL_OVERLAP", "1") == "1"

# layer table: (n_kg_load, kg_ch, K, n_kg_mm, n_mg, M_total, H_in_data, H_out)
#   L1: x[256] -> stacked 288 (yr144,yi144), kgroups 2x128, mgroups 3x96
#   L2: 288 -> 192, kgroups 3x96, mgroups 2x96
#   L3, L4: 192 -> 192, kgroups 2x96, mgroups 2x96
#   L5 head: mag[96] -> 3
LAYERS = [
    dict(K=128, n_kg=2, Cin=256, n_mg=3, Mg=96, Mtot=288, Hin=69, Hout=68),
    dict(K=96, n_kg=3, Cin=288, n_mg=2, Mg=96, Mtot=192, Hin=68, Hout=67),
    dict(K=96, n_kg=2, Cin=192, n_mg=2, Mg=96, Mtot=192, Hin=67, Hout=66),
    dict(K=96, n_kg=2, Cin=192, n_mg=2, Mg=96, Mtot=192, Hin=66, Hout=65),
    dict(K=96, n_kg=2, Cin=192, n_mg=1, Mg=3, Mtot=3, Hin=65, Hout=64),
]

# packed-input offset tables (elements, C-order raveled per piece)
PB_OFF = {}
_o = 0
PB_OFF["x"] = _o
_o += 256 * 70 * WB
for _li, _L in enumerate(LAYERS):
    PB_OFF[f"w{_li + 1}"] = _o
    _o += _L["K"] * 9 * (_L["n_kg"] if _li != 4 else 1) * _L["Mtot"]
PB_TOTAL = _o
PF_OFF = {}
_o = 0
for _li in range(4):
    PF_OFF[f"gb{_li + 1}"] = _o
    _o += LAYERS[_li]["Mtot"] * 2
PF_OFF["hb"] = _o
_o += 3
PF_TOTAL = _o

_nc_cache = {}


def _dt(mode):
    return mybir.dt.bfloat16 if mode == "bf16" else mybir.dt.float32


def _npdt(mode):
    return _BF16 if mode == "bf16" else np.float32


def build_program(mode, overlap=None):
    if overlap is None:
        overlap = OVERLAP_STATS
    cache_key = (mode, overlap)
    if cache_key in _nc_cache:
        return _nc_cache[cache_key]
    DT = _dt(mode)
    F32 = mybir.dt.float32
    R = 16 if mode == "bf16" else 8  # chunk rows (SBUF-limited for f32)
    nc = bacc.Bacc("TRN2", target_bir_lowering=False, debug=False,
                   num_devices=N_CORES)

    # ---- external I/O ----
    # All inputs are packed into TWO flat tensors (one per dtype): each extra
    # input tensor costs ~80 us of per-execution buffer plumbing in the
    # runtime, so 11 separate inputs would dominate the kernel time.
    pb_ext = nc.dram_tensor("pb", [1, PB_TOTAL], DT,
                            kind="ExternalInput").ap()
    pf_ext = nc.dram_tensor("pf", [1, PF_TOTAL], F32,
                            kind="ExternalInput").ap()
    x_ext = pb_ext[0, PB_OFF["x"]:PB_OFF["x"] + 256 * 70 * WB].rearrange(
        "(c h w) -> c h w", c=256, h=70)
    w_ext = []
    for li, L in enumerate(LAYERS):
        nkg = L["n_kg"] if li != 4 else 1
        n = L["K"] * 9 * nkg * L["Mtot"]
        w_ext.append(
            pb_ext[0, PB_OFF[f"w{li + 1}"]:PB_OFF[f"w{li + 1}"] + n].rearrange(
                "(k t g m) -> k t g m", k=L["K"], t=9, g=nkg))
    gb_ext = []
    for li in range(4):
        n = LAYERS[li]["Mtot"] * 2
        gb_ext.append(
            pf_ext[0, PF_OFF[f"gb{li + 1}"]:PF_OFF[f"gb{li + 1}"] + n
                   ].rearrange("(m two) -> m two", two=2))
    hb_ext = pf_ext[0, PF_OFF["hb"]:PF_OFF["hb"] + 3].rearrange(
        "(a b) -> a b", b=1)
    out_ext = nc.dram_tensor("out", [3, OWN, W], F32,
                             kind="ExternalOutput").ap()

    FLAT = (R + 2) * WB + 2  # flat in-tile size (1 lead + rows+2 + 1 tail)

    with tile.TileContext(nc) as tc, ExitStack() as ctx:
        wpool = ctx.enter_context(tc.tile_pool(name="wts", bufs=2))
        inpool = ctx.enter_context(tc.tile_pool(name="inp", bufs=2))
        stpool = ctx.enter_context(tc.tile_pool(name="stage", bufs=3))
        pspool = ctx.enter_context(tc.tile_pool(name="ps", bufs=8, space="PSUM"))
        stats = ctx.enter_context(tc.tile_pool(name="stats", bufs=1))
        small = ctx.enter_context(tc.tile_pool(name="small", bufs=4))
        stp = ctx.enter_context(tc.tile_pool(name="stv", bufs=2))
        dram = ctx.enter_context(tc.tile_pool(name="dram", bufs=1, space="DRAM"))

        # constants
        eps_t = small.tile([128, 1], F32, tag="eps")
        nc.vector.memset(eps_t[:], BN_EPS)
        hb_t = small.tile([3, 1], F32, tag="hb")
        nc.sync.dma_start(out=hb_t[:], in_=hb_ext)

        # per-layer weights resident whole kernel
        w_t = []
        for li, L in enumerate(LAYERS):
            nkg = L["n_kg"] if li != 4 else 1
            t = wpool.tile([L["K"], 9, nkg, L["Mtot"]], DT, tag="w",
                           name=f"wt{li}")
            nc.sync.dma_start(out=t[:], in_=w_ext[li])
            w_t.append(t)

        # DRAM spill buffers for layer outputs (raw conv out, pre-BN)
        y_dram = []
        for li in range(4):
            L = LAYERS[li]
            y_dram.append(dram.tile([L["Mtot"], L["Hout"], WB], DT,
                                    tag=f"y{li}", name=f"y{li}"))
        cc_in = [dram.tile([LAYERS[li]["Mtot"], 2], F32, tag=f"cci{li}",
                           name=f"cci{li}")
                 for li in range(4)]
        cc_out = [dram.tile([LAYERS[li]["Mtot"], 2], F32, tag=f"cco{li}",
                            name=f"cco{li}")
                  for li in range(4)]

        st_cur = None  # list per kgroup of [96,2] tiles (s=col0, t=col1)

        def emit_stats(li, L, stat_t):
            """Aggregate BN stats, AllReduce, and build scale/shift for the
            next layer's kgroups. Issued right after the last owned-row chunk
            so the collective overlaps the halo-row matmuls."""
            Mg = L["Mg"]
            for mg in range(L["n_mg"]):
                m0 = mg * Mg
                mv = small.tile([Mg, 2], F32, tag="mv")
                nc.vector.bn_aggr(out=mv[:], in_=stat_t[mg][:])
                sums = small.tile([Mg, 2], F32, tag="sums")
                nc.vector.tensor_scalar_mul(
                    out=sums[:, 0:1], in0=mv[:, 0:1], scalar1=CNT_LOCAL)
                sq = small.tile([Mg, 1], F32, tag="sq")
                nc.vector.tensor_mul(sq[:], mv[:, 0:1], mv[:, 0:1])
                nc.vector.tensor_add(sq[:], sq[:], mv[:, 1:2])
                nc.vector.tensor_scalar_mul(
                    out=sums[:, 1:2], in0=sq[:], scalar1=CNT_LOCAL)
                nc.sync.dma_start(out=cc_in[li][m0:m0 + Mg, :], in_=sums[:])
            nc.gpsimd.collective_compute(
                "AllReduce", mybir.AluOpType.add,
                replica_groups=[list(range(N_CORES))],
                ins=[cc_in[li][:].opt()], outs=[cc_out[li][:].opt()])
            # consumer kgroups of the next layer read 96-channel slices
            nL = LAYERS[li + 1]
            st_next = []
            for kg in range(nL["n_kg"]):
                k0 = kg * 96
                sr = small.tile([96, 2], F32, tag="sr")
                nc.sync.dma_start(out=sr[:], in_=cc_out[li][k0:k0 + 96, :])
                gbt = small.tile([96, 2], F32, tag="gbt")
                nc.sync.dma_start(out=gbt[:], in_=gb_ext[li][k0:k0 + 96, :])
                mean = small.tile([96, 1], F32, tag="mean")
                nc.vector.tensor_scalar_mul(
                    out=mean[:], in0=sr[:, 0:1], scalar1=1.0 / CNT_TOTAL)
                var = small.tile([96, 1], F32, tag="var")
                nc.vector.tensor_scalar_mul(
                    out=var[:], in0=sr[:, 1:2], scalar1=1.0 / CNT_TOTAL)
                msq = small.tile([96, 1], F32, tag="msq")
                nc.vector.tensor_mul(msq[:], mean[:], mean[:])
                nc.vector.tensor_sub(var[:], var[:], msq[:])
                std = small.tile([96, 1], F32, tag="std")
                nc.scalar.activation(
                    out=std[:], in_=var[:],
                    func=mybir.ActivationFunctionType.Sqrt,
                    bias=eps_t[0:96, :])
                rstd = small.tile([96, 1], F32, tag="rstd")
                nc.vector.reciprocal(out=rstd[:], in_=std[:])
                st = stp.tile([96, 2], F32, tag=f"stv{kg}")
                nc.vector.tensor_mul(st[:, 0:1], rstd[:], gbt[:, 0:1])
                tmp2 = small.tile([96, 1], F32, tag="tmp2")
                nc.vector.tensor_mul(tmp2[:], mean[:], st[:, 0:1])
                nc.vector.tensor_sub(st[:, 1:2], gbt[:, 1:2], tmp2[:])
                st_next.append(st)
            return st_next

        for li, L in enumerate(LAYERS):
            K, Mg, Mtot, Hout, Hin = L["K"], L["Mg"], L["Mtot"], L["Hout"], L["Hin"]
            n_kg_load = L["n_kg"]
            is_head = li == 4
            n_mm_kg = 1 if is_head else n_kg_load
            n_chunks = (Hout + R - 1) // R
            # last chunk whose rows contribute BN stats (owned rows 0..63);
            # the stats chain + AllReduce issue right after it so the
            # collective overlaps the remaining halo-row chunks.
            own_last = (OWN + R - 1) // R - 1
            st_next = None

            # per-mg stats buffers [Mg, 64, 6]
            if not is_head:
                stat_t = [stats.tile([Mg, OWN, 6], F32, tag=f"sb{mg}",
                                     name=f"sb{li}_{mg}")
                          for mg in range(L["n_mg"])]

            for c in range(n_chunks):
                y0 = c * R
                rows = min(R, Hout - y0)
                used = (rows + 2) * WB  # data region size (from flat idx 1)
                tail = used + 1

                # ---- load input chunk per kgroup ----
                in_t = []
                for kg in range(n_kg_load):
                    it = inpool.tile([K if li == 0 else 96, FLAT], DT,
                                     tag=f"in{kg}")
                    nc.vector.memset(it[:, 0:1], 0.0)
                    nc.vector.memset(it[:, tail:tail + 1], 0.0)
                    if li == 0:
                        ch0 = kg * 128
                        nc.sync.dma_start(
                            out=it[:, 1:1 + used],
                            in_=x_ext[ch0:ch0 + 128, y0:y0 + rows + 2, :])
                    else:
                        ch0 = kg * 96
                        src = y_dram[li - 1]
                        if y0 == 0:
                            nc.vector.memset(it[:, 1:1 + WB], 0.0)
                            nc.sync.dma_start(
                                out=it[:, 1 + WB:1 + used],
                                in_=src[ch0:ch0 + 96, 0:rows + 1, :])
                            na, nb = 1 + WB, 1 + used
                        else:
                            nc.sync.dma_start(
                                out=it[:, 1:1 + used],
                                in_=src[ch0:ch0 + 96, y0 - 1:y0 + rows + 1, :])
                            na, nb = 1, 1 + used
                        # normalize + relu (BN of previous layer), in place
                        nc.scalar.activation(
                            out=it[:, na:nb], in_=it[:, na:nb],
                            func=mybir.ActivationFunctionType.Relu,
                            bias=st_cur[kg][:, 1:2], scale=st_cur[kg][:, 0:1])
                        # zero the W pad columns (post-normalize)
                        v3 = it[:, 1:1 + used].rearrange(
                            "p (r w) -> p r w", w=WB)
                        nc.vector.memset(v3[:, :, 0:1], 0.0)
                        nc.vector.memset(v3[:, :, WB - 1:WB], 0.0)
                    in_t.append(it)

                # ---- head: magnitude sqrt(re^2+im^2) ----
                if is_head:
                    mag = inpool.tile([96, FLAT], DT, tag="in2")
                    lim = tail + 1
                    nc.vector.tensor_mul(mag[:, 0:lim], in_t[0][:, 0:lim],
                                         in_t[0][:, 0:lim])
                    # square imag in place (it has no further readers)
                    nc.vector.tensor_mul(in_t[1][:, 0:lim], in_t[1][:, 0:lim],
                                         in_t[1][:, 0:lim])
                    nc.vector.tensor_add(mag[:, 0:lim], mag[:, 0:lim],
                                         in_t[1][:, 0:lim])
                    nc.scalar.activation(
                        out=mag[:, 0:lim], in_=mag[:, 0:lim],
                        func=mybir.ActivationFunctionType.Sqrt)
                    mm_in = [mag]
                else:
                    mm_in = in_t

                # ---- matmul tiles: 2 output rows per PSUM tile ----
                n_t = (rows + 1) // 2
                for mg in range(L["n_mg"]):
                    m0 = mg * Mg
                    stg = stpool.tile([Mg, R * WB], F32 if is_head else DT,
                                      tag="st")
                    for j in range(n_t):
                        r2 = min(2, rows - 2 * j)
                        N = r2 * WB
                        ps = pspool.tile([Mg, N], F32, tag="ps")
                        nmm = 9 * n_mm_kg
                        i_mm = 0
                        for kg in range(n_mm_kg):
                            for t in range(9):
                                dy, dx = t // 3 - 1, t % 3 - 1
                                off = 1 + (2 * j + 1 + dy) * WB + dx
                                nc.tensor.matmul(
                                    ps[:],
                                    w_t[li][:, t, kg, m0:m0 + Mg],
                                    mm_in[kg][:, off:off + N],
                                    start=(i_mm == 0), stop=(i_mm == nmm - 1))
                                i_mm += 1
                        if not is_head and y0 < OWN:
                            psv = ps[:].rearrange("p (r w) -> p r w", w=WB)
                            slot = y0 + 2 * j
                            for r in range(r2):
                                nc.vector.bn_stats(
                                    out=stat_t[mg][:, slot + r:slot + r + 1, :],
                                    in_=psv[:, r:r + 1, 1:1 + W])
                        dst = stg[:, 2 * j * WB:2 * j * WB + N]
                        if is_head:
                            nc.vector.tensor_scalar_add(
                                out=dst, in0=ps[:], scalar1=hb_t[:])
                        else:
                            nc.vector.tensor_copy(out=dst, in_=ps[:])
                    if is_head:
                        nc.scalar.activation(
                            out=stg[0:1, 0:rows * WB], in_=stg[0:1, 0:rows * WB],
                            func=mybir.ActivationFunctionType.Sigmoid)
                        sv = stg[:, 0:rows * WB].rearrange(
                            "p (r w) -> p r w", w=WB)
                        nc.sync.dma_start(
                            out=out_ext[:, y0:y0 + rows, :],
                            in_=sv[:, :, 1:1 + W])
                    else:
                        nc.sync.dma_start(
                            out=y_dram[li][m0:m0 + Mg, y0:y0 + rows, :],
                            in_=stg[:, 0:rows * WB])

                if not is_head and overlap and c == own_last:
                    st_next = emit_stats(li, L, stat_t)

            if not is_head:
                if not overlap:
                    st_next = emit_stats(li, L, stat_t)
                st_cur = st_next

    nc.compile()
    _nc_cache[cache_key] = nc
    return nc


def _prep_inputs(x, w1r, w1i, g1, b1, w2r, w2i, g2, b2,
                 w3r, w3i, g3, b3, w4r, w4i, g4, b4, wc, bc, wg, bg,
                 mode):
    """Host-side shard + pack. Returns in_maps list of 8 dicts."""
    npdt = _npdt(mode)
    x = np.asarray(x, np.float32)

    # stacked block weights [Mtot, Cin, 3, 3]
    W1 = np.concatenate([w1r, w1i], axis=0)
    def blk(wr, wi):
        top = np.concatenate([wr, -wi], axis=1)
        bot = np.concatenate([wi, wr], axis=1)
        return np.concatenate([top, bot], axis=0)
    W2, W3, W4 = blk(w2r, w2i), blk(w3r, w3i), blk(w4r, w4i)
    W5 = np.concatenate([wc, wg], axis=0)
    Ws = [W1, W2, W3, W4, W5]

    def pack_w(Wf, K, nkg, flip):
        # -> [K, 9, nkg, Mtot] with t = ky*3+kx, k-groups along Cin
        if flip:
            Wf = Wf[:, :, ::-1, :]
        Mtot, Cin = Wf.shape[0], Wf.shape[1]
        a = Wf.transpose(2, 3, 1, 0).reshape(9, Cin, Mtot)  # [t, cin, m]
        a = a.reshape(9, nkg, K, Mtot).transpose(2, 0, 1, 3)  # [K,9,g,M]
        return np.ascontiguousarray(a, dtype=npdt)

    gbs = []
    for g, b in ((g1, b1), (g2, b2), (g3, b3), (g4, b4)):
        gs = np.concatenate([g, g]).astype(np.float32)
        bs = np.concatenate([b, b]).astype(np.float32)
        gbs.append(np.ascontiguousarray(np.stack([gs, bs], axis=1)))
    hb = np.concatenate([bc, bg]).astype(np.float32).reshape(3, 1)

    pf = np.empty((1, PF_TOTAL), np.float32)
    for li in range(4):
        o = PF_OFF[f"gb{li + 1}"]
        pf[0, o:o + gbs[li].size] = gbs[li].ravel()
    pf[0, PF_OFF["hb"]:PF_OFF["hb"] + 3] = hb.ravel()

    # weight packs differ only by the vertical flip; build both once
    wpacks = {}
    for flip in (False, True):
        pieces = []
        for li, L in enumerate(LAYERS):
            nkg = L["n_kg"] if li != 4 else 1
            pieces.append(pack_w(Ws[li], L["K"], nkg, flip=flip).ravel())
        wpacks[flip] = np.concatenate(pieces)

    in_maps = []
    for core in range(N_CORES):
        b_idx, h = core // 2, core % 2
        xi = x[b_idx]
        if h == 1:
            xi = xi[:, ::-1, :]
        # x block [256, 70, WB]: row 0 zero (local -1), rows 1..69 = local 0..68
        xs = np.zeros((256, 70, WB), np.float32)
        xs[:, 1:70, 1:1 + W] = xi[:, 0:69, :]
        pb = np.empty((1, PB_TOTAL), npdt)
        pb[0, PB_OFF["x"]:PB_OFF["x"] + xs.size] = \
            xs.astype(npdt).ravel()
        pb[0, PB_OFF["w1"]:] = wpacks[h == 1]
        in_maps.append({"pb": pb, "pf": pf})
    return in_maps


_runner_cache = {}


def _get_runner(mode):
    """Build the SPMD jit executable once; returns run(in_maps) -> list of
    per-core output dicts. Mirrors bass2jax.run_bass_via_pjrt but caches the
    jitted callable so repeated kernel() calls don't re-trace/re-compile."""
    if mode in _runner_cache:
        return _runner_cache[mode]
    import jax
    from concourse import bass2jax
    from jax.experimental.shard_map import shard_map
    from jax.sharding import Mesh, PartitionSpec

    nc = build_program(mode)
    bass2jax.install_neuronx_cc_hook()

    partition_name = (nc.partition_id_tensor.name
                      if nc.partition_id_tensor else None)
    in_names, out_names, out_avals, zero_outs = [], [], [], []
    for alloc in nc.m.functions[0].allocations:
        if not isinstance(alloc, mybir.MemoryLocationSet):
            continue
        name = alloc.memorylocations[0].name
        if alloc.kind == "ExternalInput":
            if name != partition_name:
                in_names.append(name)
        elif alloc.kind == "ExternalOutput":
            shape = tuple(alloc.tensor_shape)
            dtype = mybir.dt.np(alloc.dtype)
            out_names.append(name)
            out_avals.append(jax.core.ShapedArray(shape, dtype))
            zero_outs.append(np.zeros(shape, dtype))
    n_params, n_outs = len(in_names), len(out_avals)
    all_names = list(in_names + out_names)
    if partition_name is not None:
        all_names.append(partition_name)
    all_names = tuple(all_names)
    donate = tuple(range(n_params, n_params + n_outs))

    def _body(*args):
        operands = list(args)
        if partition_name is not None:
            operands.append(bass2jax.partition_id_tensor())
        outs = bass2jax._bass_exec_p.bind(
            *operands,
            out_avals=tuple(out_avals),
            in_names=all_names,
            out_names=tuple(out_names),
            lowering_input_output_aliases=(),
            sim_require_finite=True,
            sim_require_nnan=True,
            nc=nc,
        )
        return tuple(outs)

    devices = jax.devices()[:N_CORES]
    mesh = Mesh(np.asarray(devices), ("core",))
    in_specs = (PartitionSpec("core"),) * (n_params + n_outs)
    out_specs = (PartitionSpec("core"),) * n_outs
    sharded = jax.jit(
        shard_map(_body, mesh=mesh, in_specs=in_specs, out_specs=out_specs,
                  check_rep=False),
        donate_argnums=donate, keep_unused=True)

    def run(in_maps):
        concat_in = [
            np.concatenate([np.asarray(in_maps[c][nm]) for c in
                            range(N_CORES)], axis=0)
            for nm in in_names
        ]
        concat_zeros = [
            np.zeros((N_CORES * z.shape[0], *z.shape[1:]), z.dtype)
            for z in zero_outs
        ]
        out_arrs = sharded(*concat_in, *concat_zeros)
        return [
            {nm: np.asarray(out_arrs[i]).reshape(N_CORES, *out_avals[i].shape)[c]
             for i, nm in enumerate(out_names)}
            for c in range(N_CORES)
        ]

    def time_device(in_maps, reps=5):
        """Time executions with inputs pre-staged on device (excludes host
        prep and host->device transfer). Returns list of seconds. NOTE: a
        single blocking dispatch through the axon tunnel carries ~80-90 ms
        of fixed round-trip latency that is NOT device execution time."""
        import time as _time
        from jax.sharding import NamedSharding
        concat_in = [
            np.concatenate([np.asarray(in_maps[c][nm]) for c in
                            range(N_CORES)], axis=0)
            for nm in in_names
        ]
        sh = NamedSharding(mesh, PartitionSpec("core"))
        dev_in = [jax.device_put(a, sh) for a in concat_in]
        for a in dev_in:
            a.block_until_ready()
        times = []
        for _ in range(reps):
            concat_zeros = [
                jax.device_put(
                    np.zeros((N_CORES * z.shape[0], *z.shape[1:]), z.dtype),
                    sh)
                for z in zero_outs
            ]
            for a in concat_zeros:
                a.block_until_ready()
            t0 = _time.time()
            out_arrs = sharded(*dev_in, *concat_zeros)
            for o in out_arrs:
                o.block_until_ready()
            times.append(_time.time() - t0)
        return times

    def time_device_pipelined(in_maps, Ks=(2, 34), reps=4):
        """Measure the true per-execution device time by chaining K identical
        executions (call i's outputs seed call i+1's donated output operands,
        creating a data dependency so executions run back-to-back on the
        device) and taking the wall-clock slope between two chain lengths.
        This amortizes the axon-tunnel round-trip latency (~85 ms, paid once
        per chain) to zero; the slope is the steady-state per-execution
        hardware time. Returns (t_exec_seconds, {K: best_wall_seconds})."""
        import time as _time
        from jax.sharding import NamedSharding
        concat_in = [
            np.concatenate([np.asarray(in_maps[c][nm]) for c in
                            range(N_CORES)], axis=0)
            for nm in in_names
        ]
        sh = NamedSharding(mesh, PartitionSpec("core"))
        dev_in = [jax.device_put(a, sh) for a in concat_in]
        for a in dev_in:
            a.block_until_ready()

        def make_zeros():
            return [
                jax.device_put(
                    np.zeros((N_CORES * z.shape[0], *z.shape[1:]), z.dtype),
                    sh)
                for z in zero_outs
            ]

        # warm-up execution
        outs = tuple(sharded(*dev_in, *make_zeros()))
        for o in outs:
            o.block_until_ready()
        walls = {}
        for K in sorted(Ks):
            ts = []
            for _ in range(reps):
                outs = tuple(make_zeros())
                for o in outs:
                    o.block_until_ready()
                t0 = _time.time()
                for _ in range(K):
                    outs = sharded(*dev_in, *outs)
                for o in outs:
                    o.block_until_ready()
                ts.append(_time.time() - t0)
            walls[K] = min(ts)
        K1, K2 = min(Ks), max(Ks)
        t_exec = (walls[K2] - walls[K1]) / (K2 - K1)
        return t_exec, walls

    run.time_device = time_device
    run.time_device_pipelined = time_device_pipelined
    _runner_cache[mode] = run
    return run


def kernel(**inputs):
    mode = DT_MODE
    run = _get_runner(mode)
    in_maps = _prep_inputs(mode=mode, **inputs)
    results = run(in_maps)
    out = np.zeros((4, 3, H, W), np.float32)
    for core in range(N_CORES):
        b_idx, h = core // 2, core % 2
        oc = results[core]["out"]  # [3, 64, W]
        if h == 0:
            out[b_idx, :, 0:OWN, :] = oc
        else:
            out[b_idx, :, OWN:H, :] = oc[:, ::-1, :]
    return out

